# revision 1
# baseline (speedup 1.0000x reference)
# MoE (top-2 of 8 experts, SwiGLU FFN) on 8 trn2 NeuronCores.
#
# Sharding strategy (expert-parallel, routing-aware):
#   The host computes the router (gate logits -> softmax -> top-2 ->
#   renormalized combine weights; ~0.01% of total FLOPs) and uses the
#   routing decision to SHARD the tokens: core e receives exactly the
#   tokens routed to expert e (padded to a common capacity C), plus
#   expert e's weights pre-transposed for the PE's lhsT layout.  Each
#   core runs the fused SwiGLU FFN ( silu(x@w1T) * (x@w3T) ) @ w2T over
#   its token slice in bf16 with fp32 PSUM accumulation.  The host then
#   scatter-adds each expert's output back with the combine weights.
#   This does 2/8 of the dense all-experts compute (only routed tokens),
#   the sharding hint's "all-to-all dispatch/combine" realized as the
#   host-side gather/scatter that sharding full inputs requires anyway.
#
# Device layout: everything is token-minor ("transposed"): activations
# live as [feature, token] so the contraction dim (features) sits on
# SBUF partitions for both matmul operands and no on-device transposes
# are needed.

import math
import sys

import numpy as np

if "/opt/trn_rl_repo" not in sys.path:
    sys.path.insert(0, "/opt/trn_rl_repo")

import concourse.bass as bass
import concourse.mybir as mybir
import concourse.tile as tile
from concourse.bass_utils import run_bass_kernel_spmd

B, S, H, I, E, TOPK = 4, 2048, 1024, 4096, 8, 2
T = B * S
P = 128
HC = H // P   # 8 h-chunks
IC = I // P   # 32 i-chunks
TB = 512      # token block (PSUM bank = 512 fp32)
OCG = 4       # output h'-chunks accumulated concurrently (PSUM banks)

_BF16 = mybir.dt.np(mybir.dt.bfloat16)

_PROGRAM_CACHE: dict = {}
_LAST_IN_MAPS = None

# This toolchain's walrus enforces small per-ISA-struct sync-wait budgets
# (a DVE TensorTensor takes ONE wait; the Tile-exit Drain takes one, etc.).
# Tile attaches as many waits as deps require, so we legalize post-hoc:
# hoist excess waits into standalone EventSemaphore instructions inserted
# immediately before the offending instruction on the same engine queue —
# semantically identical to carrying the wait on the instruction itself.
_WAIT_BUDGET: dict = {}
_DEFAULT_WAIT_BUDGET = 1


def _legalize_sync_waits(nc):
    import json as _json

    m = _json.loads(nc.to_json_bytes())
    n_new = 0
    for fn in m["functions"]:
        for blk in fn["blocks"]:
            out = []
            for inst in blk["instructions"]:
                si = inst.get("sync_info")
                waits = (si or {}).get("on_wait") or []
                budget = _WAIT_BUDGET.get(inst.get("opcode"),
                                          _DEFAULT_WAIT_BUDGET)
                if len(waits) > budget:
                    for w in waits[:-budget]:
                        n_new += 1
                        out.append({
                            "debug": inst.get("debug", 0),
                            "engine": inst["engine"],
                            "ins": [],
                            "outs": [],
                            "name": f"I-legw-{n_new}",
                            "opcode": "EventSemaphore",
                            "sync_info": {"on_update": [], "on_wait": [w]},
                        })
                    si["on_wait"] = waits[-budget:]
                out.append(inst)
            blk["instructions"] = out
    data = _json.dumps(m).encode()
    nc.to_json_bytes = lambda: data  # shadow for bass2jax/compile paths
    return n_new


def _build_program(C: int, repeat: int = 1):
    """One SPMD Bass program: fused SwiGLU FFN over [H, C] tokens.

    repeat > 1 wraps the whole token loop in a hardware For_i loop —
    used only by the benchmark harness to amortize host/axon dispatch
    overhead out of wall-clock timings.
    """
    dt_in = mybir.dt.bfloat16
    f32 = mybir.dt.float32
    nc = bass.Bass()
    xgT = nc.dram_tensor("xgT", [H, C], dt_in, kind="ExternalInput")
    w1T = nc.dram_tensor("w1T", [H, I], dt_in, kind="ExternalInput")
    w3T = nc.dram_tensor("w3T", [H, I], dt_in, kind="ExternalInput")
    w2T = nc.dram_tensor("w2T", [I, H], dt_in, kind="ExternalInput")
    ygT = nc.dram_tensor("ygT", [H, C], f32, kind="ExternalOutput")

    NB = math.ceil(C / TB)
    xgT_r = xgT.rearrange("(hc p) c -> p hc c", p=P)
    ygT_r = ygT.rearrange("(oc p) c -> p oc c", p=P)

    with tile.TileContext(nc) as tc:
        with (
            tc.tile_pool(name="w13", bufs=1) as wpool,
            tc.tile_pool(name="xg", bufs=2) as xpool,
            tc.tile_pool(name="g", bufs=1) as gpool,
            tc.tile_pool(name="w2", bufs=6) as w2pool,
            tc.tile_pool(name="act", bufs=2) as spool,
            tc.tile_pool(name="ot", bufs=3) as opool,
            tc.tile_pool(name="psA", bufs=2, space="PSUM") as psA,
            tc.tile_pool(name="psB", bufs=max(1, 4 // OCG), space="PSUM") as psB,
        ):
            # Prefetch the first token block ahead of the (large) weight
            # preload so stage A's first matmuls aren't gated on 17 MB.
            n0 = _block_sizes(C)[0]
            xg0 = xpool.tile([P, HC, TB], dt_in, tag="xg")
            nc.sync.dma_start(xg0[:, :, :n0], xgT_r[:, :, 0:n0])

            # w1T/w3T stay resident in SBUF for the whole kernel (16 MB
            # bf16), DMA'd in column chunks ordered to match stage A's
            # ic-ascending consumption (w1 cols then w3 cols).
            w1t = [wpool.tile([P, I], dt_in, tag=f"w1_{hc}", name=f"w1_{hc}")
                   for hc in range(HC)]
            w3t = [wpool.tile([P, I], dt_in, tag=f"w3_{hc}", name=f"w3_{hc}")
                   for hc in range(HC)]
            WCH = 512
            for c0 in range(0, I, WCH):
                for hc in range(HC):
                    nc.sync.dma_start(w1t[hc][:, c0:c0 + WCH],
                                      w1T[hc * P:(hc + 1) * P, c0:c0 + WCH])
                for hc in range(HC):
                    nc.sync.dma_start(w3t[hc][:, c0:c0 + WCH],
                                      w3T[hc * P:(hc + 1) * P, c0:c0 + WCH])

            from contextlib import nullcontext
            rep_ctx = tc.For_i(0, repeat, 1) if repeat > 1 else nullcontext()
            with rep_ctx:
                _token_loop(nc, tc, C, dt_in, f32,
                            xgT_r, ygT_r, w2T, w1t, w3t, xg0,
                            xpool, gpool, w2pool, spool, opool, psA, psB)
    return nc


def _block_sizes(C):
    """Split C into ceil(C/TB) near-equal blocks (multiples of 32, <=TB).
    Even blocks avoid a tiny tail block whose per-op fixed overheads on
    ACT/DVE stall the PE; total matmul issue cycles are identical."""
    NB = math.ceil(C / TB)
    per, extra = divmod(C // 32, NB)
    return [(per + (1 if i < extra else 0)) * 32 for i in range(NB)]


def _token_loop(nc, tc, C, dt_in, f32, xgT_r, ygT_r, w2T, w1t, w3t, xg0,
                xpool, gpool, w2pool, spool, opool, psA, psB):
    sizes = _block_sizes(C)
    offs = [sum(sizes[:i]) for i in range(len(sizes))]
    NB = len(sizes)
    xg_tiles = {0: xg0}
    for tb in range(NB):
        n, o = sizes[tb], offs[tb]
        # Prefetch the NEXT block's tokens now, ahead of this block's w2
        # stream on the DMA queue — otherwise xg arrives late and the PE
        # stalls a few us at every block boundary.
        if tb + 1 < NB:
            nxt = xpool.tile([P, HC, TB], dt_in, tag="xg")
            nc.sync.dma_start(nxt[:, :, :sizes[tb + 1]],
                              xgT_r[:, :, offs[tb + 1]:offs[tb + 1] + sizes[tb + 1]])
            xg_tiles[tb + 1] = nxt
        # (Block 0 was prefetched before the weight preload; in
        # repeat/benchmark mode later iterations see stale data in that
        # slot — values are irrelevant for timing.)
        xg = xg_tiles.pop(tb)

        # Stage A: g = silu(x@w1T) * (x@w3T), kept on-chip as bf16.
        g = gpool.tile([P, IC, TB], dt_in, tag="g")
        for ic in range(IC):
            isl = slice(ic * P, (ic + 1) * P)
            ps1 = psA.tile([P, TB], f32, tag="ps1")
            ps3 = psA.tile([P, TB], f32, tag="ps3")
            for hc in range(HC):
                nc.tensor.matmul(ps1[:, :n], w1t[hc][:, isl], xg[:, hc, :n],
                                 start=(hc == 0), stop=(hc == HC - 1))
            for hc in range(HC):
                nc.tensor.matmul(ps3[:, :n], w3t[hc][:, isl], xg[:, hc, :n],
                                 start=(hc == 0), stop=(hc == HC - 1))
            sl = spool.tile([P, TB], f32, tag="silu")
            nc.scalar.activation(sl[:, :n], ps1[:, :n],
                                 mybir.ActivationFunctionType.Silu)
            nc.vector.tensor_mul(out=g[:, ic, :n], in0=sl[:, :n],
                                 in1=ps3[:, :n])

        # Stage B: ygT = g.T-contracted with w2T, OCG output chunks
        # at a time so PSUM banks cycle while groups copy out.
        for grp in range(HC // OCG):
            pst = [psB.tile([P, TB], f32, tag=f"psB{j}", name=f"psB{j}")
                   for j in range(OCG)]
            csl = slice(grp * OCG * P, (grp + 1) * OCG * P)
            for ic in range(IC):
                w2tile = w2pool.tile([P, OCG * P], dt_in, tag="w2")
                nc.sync.dma_start(w2tile[:], w2T[ic * P:(ic + 1) * P, csl])
                for j in range(OCG):
                    nc.tensor.matmul(pst[j][:, :n],
                                     w2tile[:, j * P:(j + 1) * P],
                                     g[:, ic, :n],
                                     start=(ic == 0), stop=(ic == IC - 1))
            for j in range(OCG):
                ot = opool.tile([P, TB], f32, tag="ot")
                nc.vector.tensor_copy(ot[:, :n], pst[j][:, :n])
                nc.sync.dma_start(ygT_r[:, grp * OCG + j, o:o + n],
                                  ot[:, :n])


def _route(xt: np.ndarray, Wg: np.ndarray):
    """Host router: softmax over gate logits, top-2, renormalized weights."""
    logits = xt @ Wg.T.astype(np.float32)                       # [T, E]
    logits = logits - logits.max(axis=1, keepdims=True)
    p = np.exp(logits, dtype=np.float32)
    p /= p.sum(axis=1, keepdims=True)
    r = np.arange(T)
    top1 = p.argmax(axis=1)
    p2 = p.copy()
    p2[r, top1] = -1.0
    top2 = p2.argmax(axis=1)
    v1 = p[r, top1]
    v2 = p[r, top2]
    den = v1 + v2
    v1 = v1 / den
    v2 = v2 / den
    idxs, wts = [], []
    for e in range(E):
        m1 = top1 == e
        sel = np.nonzero(m1 | (top2 == e))[0]
        idxs.append(sel)
        wts.append(np.where(m1, v1, v2)[sel].astype(np.float32))
    return idxs, wts


def _run_with_retry(nc, in_maps, core_ids, attempts=4):
    """The axon-tunneled NeuronCores intermittently report
    NRT_EXEC_UNIT_UNRECOVERABLE right after a previous process used them;
    a fresh PJRT client after a cool-down recovers.  Retry transparently."""
    import time as _time

    for k in range(attempts):
        try:
            return run_bass_kernel_spmd(nc, in_maps, core_ids).results
        except Exception:
            if k == attempts - 1:
                raise
            try:
                import jax.extend as _jex
                _jex.backend.clear_backends()
            except Exception:
                pass
            _time.sleep(60 * (k + 1))


def kernel(x, Wg, w1, w3, w2):
    xt = np.ascontiguousarray(np.asarray(x, dtype=np.float32).reshape(T, H))
    idxs, wts = _route(xt, np.asarray(Wg, dtype=np.float32))
    counts = [len(ix) for ix in idxs]
    C = max(32, ((max(counts) + 31) // 32) * 32)

    key = C
    if key in _PROGRAM_CACHE:
        nc = _PROGRAM_CACHE[key]
    else:
        nc = _build_program(C)
        _legalize_sync_waits(nc)
        _PROGRAM_CACHE[key] = nc

    # Per-core inputs: gathered tokens + this expert's weights, all
    # pre-transposed on the host so every device matmul operand is a
    # natural (contiguous-row) DMA.
    w1_ = np.asarray(w1)
    w3_ = np.asarray(w3)
    w2_ = np.asarray(w2)
    in_maps = []
    for e in range(E):
        xg = np.zeros((H, C), dtype=_BF16)
        xg[:, :counts[e]] = xt[idxs[e]].T.astype(_BF16)
        in_maps.append({
            "xgT": xg,
            "w1T": np.ascontiguousarray(w1_[e].T).astype(_BF16),
            "w3T": np.ascontiguousarray(w3_[e].T).astype(_BF16),
            "w2T": np.ascontiguousarray(w2_[e].T).astype(_BF16),
        })

    global _LAST_IN_MAPS
    _LAST_IN_MAPS = in_maps
    results = _run_with_retry(nc, in_maps, list(range(E)))

    out = np.zeros((T, H), dtype=np.float32)
    for e in range(E):
        yg = results[e]["ygT"]                       # [H, C] fp32
        out[idxs[e]] += wts[e][:, None] * yg[:, :counts[e]].T
    return out.reshape(B, S, H)

